# revision 8
# baseline (speedup 1.0000x reference)
"""Balanced BCE loss with per-sample dynamic top-k negative mining on 8 TRN2 cores.

Math: for each sample the reference computes
    pos_count = sum(gt*mask), neg_raw = sum((1-gt)*mask)
    neg_count = min(neg_raw, 3*pos_count), k = int(neg_count)
    loss = BCE(pred, gt);  pos_loss = sum(loss*positive)
    neg_topk = sum of k largest loss*negative values
    per_sample = (pos_loss + neg_topk) / (pos_count + neg_count + eps); mean over N.

Every negative position has loss > 0 (p is bounded away from {0,1}), so
whenever neg_raw <= 3*pos_count the top-k sum equals the FULL sum of negative
losses and the per-sample loss reduces to scalars the device can produce with
pure streaming reductions:
    A = sum(gt*mask)   M = sum(mask)   (B = M - A)
    S = sum(gm*ln(p)) + sum((mask-gm)*ln(1-p))        (= C + D; only the SUM
                                                       is ever needed)
If a sample violates neg_raw <= 3*pos_count the host recomputes it exactly.

Device mapping: data-parallel over N, 2 samples/core.  Each [640,640] sample
is a [128, 3200] view (12800B contiguous per partition), streamed in free-dim
chunks.  Per chunk:
  - ScalarE: m16 = bf16 copy-cast of mask with accum_out -> M column (exact);
    lp = Ln(p); l1p = Ln(1-p) (activation scale/bias), all bf16 out.
  - VectorE: gm = gt*mask via tensor_tensor_reduce (f32 in, bf16 out, 1x mode
    either way) whose accum_out yields the A column for free; then 2x-mode
    bf16 tensor_tensors nm = m16-gm, t1 = gm*lp, t2 = nm*l1p.
  - TensorE: reduces t1/t2 with a stationary ones[128,1] bf16 vector into ONE
    [1,400] PSUM accumulator per sample (C+D together).
  - The final 400-wide chunk of the last sample bypasses TensorE entirely:
    t1/t2 are tensor_tensor_reduces whose accum_out lands in stats columns,
    keeping the post-DMA tail short.
Input DMAs (pred, mask, gt per chunk) are triggered from SP; output DMAs are
triggered from ScalarE (also an HWDGE engine on TRN2) so SP's in-order stream
is never blocked behind compute.  Host sums the per-partition/per-chunk
partials in float64; M and A are exact integers (0/1 tensors, f32 accums),
so B = M - A is exact and the fast/fallback decision is robust.  Only
ln values are rounded to bf16 (~2^-9 relative, averaging out over ~100k
summed elements per sample).
"""

import os
import sys

# defensive: if a previous process left a NeuronCore wedged, ask NRT to
# reset cores at init (read before first jax/NRT touch; harmless otherwise)
os.environ.setdefault("NEURON_RT_RESET_CORES", "1")

if "/opt/trn_rl_repo" not in sys.path:
    sys.path.insert(0, "/opt/trn_rl_repo")

import numpy as np

N, H, W = 16, 640, 640
NEG_RATIO = 3.0
EPS = 1e-8
N_CORES = 8
S = N // N_CORES          # samples per core
P = 128
FREE = H * W // P         # 3200
# per-sample free-dim chunk plans; the last sample ends with a small chunk
# so the final DMA->compute dependency chain is as short as possible
CHUNK_PLANS = ((1600, 1600), (1600, 1200, 400))
MM = 400                  # matmul sub-chunk (PSUM bank: <=512 f32)
# stats columns per sample: [M, A] per chunk, plus [C, D] for the PE-bypass
# final chunk of the last sample
NCOLS = tuple(2 * len(p) + (2 if s == S - 1 else 0)
              for s, p in enumerate(CHUNK_PLANS))

_STATE = {}


def _build():
    import concourse.bass as bass
    import concourse.tile as tile
    from concourse import bacc, mybir

    f32 = mybir.dt.float32
    bf16 = mybir.dt.bfloat16
    Alu = mybir.AluOpType
    Act = mybir.ActivationFunctionType
    Ax = mybir.AxisListType

    nc = bacc.Bacc("TRN2", target_bir_lowering=False, debug=False,
                   num_devices=N_CORES)
    pred_d = nc.dram_tensor("pred", [S, H, W], f32, kind="ExternalInput").ap()
    gt_d = nc.dram_tensor("gt", [S, H, W], f32, kind="ExternalInput").ap()
    mask_d = nc.dram_tensor("mask", [S, H, W], f32, kind="ExternalInput").ap()
    stats0_d = nc.dram_tensor("stats0", [P, NCOLS[0]], f32,
                              kind="ExternalOutput").ap()
    stats1_d = nc.dram_tensor("stats1", [P, NCOLS[1]], f32,
                              kind="ExternalOutput").ap()
    cd_d = nc.dram_tensor("cd", [S, MM], f32, kind="ExternalOutput").ap()

    with tile.TileContext(nc) as tc:
        with tc.tile_pool(name="cst", bufs=1) as cst, \
             tc.tile_pool(name="inp", bufs=3) as inp, \
             tc.tile_pool(name="mid", bufs=2) as mid, \
             tc.tile_pool(name="res", bufs=1) as res, \
             tc.tile_pool(name="ps", bufs=2, space="PSUM") as psp:
            ones = cst.tile([P, 1], bf16)
            nc.gpsimd.memset(ones[:], 1.0)
            stats0 = res.tile([P, NCOLS[0]], f32)
            stats1 = res.tile([P, NCOLS[1]], f32)
            stats = (stats0, stats1)
            deferred_outs = []

            for s in range(S):
                pred_v = pred_d[s].rearrange("(p a) w -> p (a w)", p=P)
                gt_v = gt_d[s].rearrange("(p a) w -> p (a w)", p=P)
                mask_v = mask_d[s].rearrange("(p a) w -> p (a w)", p=P)
                acc = psp.tile([1, MM], f32, tag="acc", name=f"acc_{s}")
                CHUNKS = CHUNK_PLANS[s]
                # matmul steps for this sample (PE-bypass chunks excluded)
                pe_chunks = [c for c, ch in enumerate(CHUNKS)
                             if not (s == S - 1 and c == len(CHUNKS) - 1)]
                nsteps = 2 * sum(CHUNKS[c] // MM for c in pe_chunks)
                off = 0
                step = 0
                for c, CH in enumerate(CHUNKS):
                    sl = slice(off, off + CH)
                    off += CH
                    bypass_pe = s == S - 1 and c == len(CHUNKS) - 1
                    tp = inp.tile([P, CH], f32, tag="pred",
                                  name=f"tp_{s}_{c}")
                    tm = inp.tile([P, CH], f32, tag="mask",
                                  name=f"tm_{s}_{c}")
                    tg = inp.tile([P, CH], f32, tag="gt", name=f"tg_{s}_{c}")
                    nc.sync.dma_start(tp[:], pred_v[:, sl])
                    nc.sync.dma_start(tm[:], mask_v[:, sl])
                    nc.sync.dma_start(tg[:], gt_v[:, sl])

                    st = stats[s]
                    jM = 2 * c
                    # lp/l1p first: pred is the first DMA of the chunk, so
                    # ScalarE starts before mask/gt have landed
                    lp = mid.tile([P, CH], bf16, tag="lp", name=f"lp_{s}_{c}")
                    nc.scalar.activation(lp[:], tp[:], Act.Ln)
                    l1p = mid.tile([P, CH], bf16, tag="l1p",
                                   name=f"l1p_{s}_{c}")
                    nc.scalar.activation(l1p[:], tp[:], Act.Ln,
                                         bias=1.0, scale=-1.0)
                    # bf16 "cast" of the 0/1 mask as ln((e-1)*x + 1),
                    # which is exactly 0->0, 1->1 -- keeps every ScalarE op
                    # in one activation-table set; accum gives sum(mask)
                    m16 = mid.tile([P, CH], bf16, tag="m16",
                                   name=f"m16_{s}_{c}")
                    nc.scalar.activation(m16[:], tm[:], Act.Ln,
                                         bias=1.0, scale=float(np.e - 1.0),
                                         accum_out=st[:, jM:jM + 1])
                    # gm = gt*mask on the otherwise-idle GpSimd engine
                    # (f32 tensor_tensor is 1x on VectorE anyway, and this
                    # keeps VectorE under the DMA pace); the 400-wide tail
                    # chunk stays on VectorE for the shortest last-chunk
                    # dependency chain
                    gm = mid.tile([P, CH], bf16, tag="gm", name=f"gm_{s}_{c}")
                    if bypass_pe:
                        nc.vector.tensor_tensor(gm[:], tg[:], tm[:], Alu.mult)
                    else:
                        nc.gpsimd.tensor_tensor(gm[:], tg[:], tm[:], Alu.mult)
                    # pos_count column: free-dim row sums of gm
                    nc.vector.tensor_reduce(st[:, jM + 1:jM + 2], gm[:],
                                            Ax.X, Alu.add)
                    nm = mid.tile([P, CH], bf16, tag="nm",
                                  name=f"nm_{s}_{c}")
                    nc.vector.tensor_tensor(nm[:], m16[:], gm[:],
                                            Alu.subtract)
                    t1 = mid.tile([P, CH], bf16, tag="t1", name=f"t1_{s}_{c}")
                    t2 = mid.tile([P, CH], bf16, tag="t2", name=f"t2_{s}_{c}")
                    nc.vector.tensor_tensor(t1[:], gm[:], lp[:], Alu.mult)
                    nc.vector.tensor_tensor(t2[:], nm[:], l1p[:], Alu.mult)
                    if bypass_pe:
                        # short-tail path: row sums on VectorE, no PE/PSUM
                        jC = 2 * len(CHUNKS)
                        nc.vector.tensor_reduce(st[:, jC:jC + 1], t1[:],
                                                Ax.X, Alu.add)
                        nc.vector.tensor_reduce(st[:, jC + 1:jC + 2], t2[:],
                                                Ax.X, Alu.add)
                    else:
                        for m in range(CH // MM):
                            nc.tensor.matmul(acc[:], ones[:],
                                             t1[:, bass.ts(m, MM)],
                                             start=step == 0,
                                             stop=step == nsteps - 1)
                            step += 1
                            nc.tensor.matmul(acc[:], ones[:],
                                             t2[:, bass.ts(m, MM)],
                                             start=step == 0,
                                             stop=step == nsteps - 1)
                            step += 1

                # PSUM -> SBUF copy now; the output DMA triggers are
                # deferred so SP's in-order stream never blocks later
                # input DMAs behind this sample's compute chain
                cd_sb = res.tile([1, MM], f32, tag=f"cd{s}", name=f"cd_sb{s}")
                if s == 0:
                    nc.vector.tensor_copy(cd_sb[:], acc[:])
                else:
                    nc.scalar.copy(cd_sb[:], acc[:])
                deferred_outs.append((cd_d[s], cd_sb))

            deferred_outs.append((stats0_d[:], stats0))
            deferred_outs.append((stats1_d[:], stats1))
            for dst, src in deferred_outs:
                nc.sync.dma_start(dst, src[:])
    nc.compile()
    return nc


def _get_nc():
    if "nc" not in _STATE:
        _STATE["nc"] = _build()
    return _STATE["nc"]


def _host_topk_fallback(p, g, m):
    """Exact per-sample reference semantics in numpy (rare path)."""
    p = p.astype(np.float32)
    positive = g * m
    negative = (1.0 - g) * m
    pos_count = positive.sum(dtype=np.float64)
    neg_count = min(negative.sum(dtype=np.float64), pos_count * NEG_RATIO)
    log_p = np.maximum(np.log(p), -100.0)
    log_1mp = np.maximum(np.log1p(-p), -100.0)
    loss = -(g * log_p + (1.0 - g) * log_1mp)
    pos_loss_sum = (loss * positive).sum(dtype=np.float64)
    neg_loss = (loss * negative).ravel()
    k = int(neg_count)
    if k > 0:
        top = np.partition(neg_loss, len(neg_loss) - k)[len(neg_loss) - k:]
        neg_topk = top.sum(dtype=np.float64)
    else:
        neg_topk = 0.0
    return (pos_loss_sum + neg_topk) / (pos_count + neg_count + EPS)


def _combine(results, p, g, m):
    losses = []
    for c in range(N_CORES):
        cd = results[c]["cd"].astype(np.float64)        # [S, MM]
        for s in range(S):
            st = results[c][f"stats{s}"].astype(np.float64)
            nch = len(CHUNK_PLANS[s])
            M = st[:, 0:2 * nch:2].sum()
            A = st[:, 1:2 * nch:2].sum()
            CD = cd[s].sum()
            if s == S - 1:
                CD += st[:, 2 * nch:2 * nch + 2].sum()
            pos_count = round(A)
            neg_raw = round(M - A)
            if neg_raw <= pos_count * NEG_RATIO:
                # top-k covers every (strictly positive) negative loss
                losses.append((-CD) / (pos_count + neg_raw + EPS))
            else:
                i = c * S + s
                losses.append(_host_topk_fallback(p[i], g[i], m[i]))
    return np.float32(np.mean(losses))


def _in_maps(p, g, m):
    return [
        {"pred": p[c * S:(c + 1) * S],
         "gt": g[c * S:(c + 1) * S],
         "mask": m[c * S:(c + 1) * S]}
        for c in range(N_CORES)
    ]


def kernel(pred, gt, mask):
    from concourse import bass_utils

    p = np.ascontiguousarray(pred[:, 0], dtype=np.float32)   # [N,H,W]
    g = np.ascontiguousarray(gt, dtype=np.float32)
    m = np.ascontiguousarray(mask, dtype=np.float32)

    nc = _get_nc()
    in_maps = _in_maps(p, g, m)
    try:
        res = bass_utils.run_bass_kernel_spmd(nc, in_maps,
                                              core_ids=list(range(N_CORES)))
    except Exception:
        # one retry: transient device wedge from a prior process
        res = bass_utils.run_bass_kernel_spmd(nc, in_maps,
                                              core_ids=list(range(N_CORES)))
    return _combine(res.results, p, g, m)


# revision 9
# speedup vs baseline: 1.0794x; 1.0794x over previous
"""Balanced BCE loss with per-sample dynamic top-k negative mining on 8 TRN2 cores.

Math: for each sample the reference computes
    pos_count = sum(gt*mask), neg_raw = sum((1-gt)*mask)
    neg_count = min(neg_raw, 3*pos_count), k = int(neg_count)
    loss = BCE(pred, gt);  pos_loss = sum(loss*positive)
    neg_topk = sum of k largest loss*negative values
    per_sample = (pos_loss + neg_topk) / (pos_count + neg_count + eps); mean over N.

Every negative position has loss > 0 (p is bounded away from {0,1}), so
whenever neg_raw <= 3*pos_count the top-k sum equals the FULL sum of negative
losses and the per-sample loss reduces to scalars the device can produce with
pure streaming reductions:
    A = sum(gt*mask)   M = sum(mask)   (B = M - A)
    CD = sum(gm*ln(p)) + sum((mask-gm)*ln(1-p))
If a sample ever violates neg_raw <= 3*pos_count the host recomputes it
exactly (numpy).

Device mapping: data-parallel over N, 2 samples/core.  Each [640,640] sample
is a [128, 3200] view (12800B contiguous per partition), streamed in free-dim
chunks (small first chunk so compute starts early, small last chunk so the
post-DMA dependency tail is short).  Per chunk:
  - ScalarE: lp = Ln(p); l1p = Ln(1-p) (activation scale/bias); m16 = bf16
    "cast" of mask via Ln((e-1)*x+1) (exact 0->0, 1->1, keeps one activation
    table) whose accum_out is the per-chunk M column.
  - VectorE: gm = gt*mask (f32 in, bf16 out), nm = m16-gm, t1 = gm*lp,
    t2 = nm*l1p (2x-mode bf16 tensor_tensors).
  - A columns (sum gm): on sample 0 a ScalarE Ln-trick pass over gm with
    accum_out; on sample 1 a VectorE free-axis tensor_reduce.  This splits
    the count-reduction cost across the two engines so both stay under the
    DMA pace (~6 us per 1600-column chunk set).
  - TensorE: reduces t1/t2 with a stationary ones[128,1] bf16 vector into a
    single [1,400] PSUM accumulator per sample (C+D together -- the fast
    path never needs them separately).  The final 400-wide chunk bypasses
    the PE: VectorE tensor_reduces write C/D columns directly, keeping the
    last-chunk chain short.
GpSimd is only used for the ones[] memset: its software tensor ops contend
with VectorE for SBUF and slow DVE ~4x (measured), so no compute goes there.
Input DMAs are triggered from SP in pred,mask,gt order (ScalarE can start on
pred before gt lands); output DMAs are deferred/interleaved so SP's in-order
stream never blocks input triggers behind compute.  Host sums the
per-partition/per-chunk partials in float64; M and A are exact integers
(0/1 tensors, f32 accumulators), so B = M - A is exact and the
fast/fallback decision is robust.  Only ln values are rounded to bf16
(~2^-9 relative, averaging out over ~100k summed elements per sample).
"""

import os
import sys

# defensive: if a previous process left a NeuronCore wedged, ask NRT to
# reset cores at init (read before first jax/NRT touch; harmless otherwise)
os.environ.setdefault("NEURON_RT_RESET_CORES", "1")

if "/opt/trn_rl_repo" not in sys.path:
    sys.path.insert(0, "/opt/trn_rl_repo")

import numpy as np

N, H, W = 16, 640, 640
NEG_RATIO = 3.0
EPS = 1e-8
N_CORES = 8
S = N // N_CORES          # samples per core
P = 128
FREE = H * W // P         # 3200
CHUNK_PLANS = ((800, 1200, 1200), (1600, 1200, 400))
MM = 400                  # matmul sub-chunk (PSUM bank: <=512 f32)
# stats columns per sample: [M, A] per chunk, plus [C, D] for the PE-bypass
# final chunk of the last sample
NCOLS = tuple(2 * len(p) + (2 if s == S - 1 else 0)
              for s, p in enumerate(CHUNK_PLANS))

_STATE = {}


def _build():
    import concourse.bass as bass
    import concourse.tile as tile
    from concourse import bacc, mybir

    f32 = mybir.dt.float32
    bf16 = mybir.dt.bfloat16
    Alu = mybir.AluOpType
    Act = mybir.ActivationFunctionType
    Ax = mybir.AxisListType

    nc = bacc.Bacc("TRN2", target_bir_lowering=False, debug=False,
                   num_devices=N_CORES)
    pred_d = nc.dram_tensor("pred", [S, H, W], f32, kind="ExternalInput").ap()
    gt_d = nc.dram_tensor("gt", [S, H, W], f32, kind="ExternalInput").ap()
    mask_d = nc.dram_tensor("mask", [S, H, W], f32, kind="ExternalInput").ap()
    stats0_d = nc.dram_tensor("stats0", [P, NCOLS[0]], f32,
                              kind="ExternalOutput").ap()
    stats1_d = nc.dram_tensor("stats1", [P, NCOLS[1]], f32,
                              kind="ExternalOutput").ap()
    cd_d = nc.dram_tensor("cd", [S, MM], f32, kind="ExternalOutput").ap()

    with tile.TileContext(nc) as tc:
        with tc.tile_pool(name="cst", bufs=1) as cst, \
             tc.tile_pool(name="inp", bufs=3) as inp, \
             tc.tile_pool(name="mid", bufs=2) as mid, \
             tc.tile_pool(name="res", bufs=1) as res, \
             tc.tile_pool(name="ps", bufs=2, space="PSUM") as psp:
            ones = cst.tile([P, 1], bf16)
            nc.gpsimd.memset(ones[:], 1.0)
            stats0 = res.tile([P, NCOLS[0]], f32)
            stats1 = res.tile([P, NCOLS[1]], f32)
            stats = (stats0, stats1)

            for s in range(S):
                pred_v = pred_d[s].rearrange("(p a) w -> p (a w)", p=P)
                gt_v = gt_d[s].rearrange("(p a) w -> p (a w)", p=P)
                mask_v = mask_d[s].rearrange("(p a) w -> p (a w)", p=P)
                acc = psp.tile([1, MM], f32, tag="acc", name=f"acc_{s}")
                CHUNKS = CHUNK_PLANS[s]
                pe_chunks = [c for c, ch in enumerate(CHUNKS)
                             if not (s == S - 1 and c == len(CHUNKS) - 1)]
                nsteps = 2 * sum(CHUNKS[c] // MM for c in pe_chunks)
                off = 0
                step = 0
                for c, CH in enumerate(CHUNKS):
                    sl = slice(off, off + CH)
                    off += CH
                    bypass_pe = s == S - 1 and c == len(CHUNKS) - 1
                    tp = inp.tile([P, CH], f32, tag="pred",
                                  name=f"tp_{s}_{c}")
                    tm = inp.tile([P, CH], f32, tag="mask",
                                  name=f"tm_{s}_{c}")
                    tg = inp.tile([P, CH], f32, tag="gt", name=f"tg_{s}_{c}")
                    nc.sync.dma_start(tp[:], pred_v[:, sl])
                    nc.sync.dma_start(tm[:], mask_v[:, sl])
                    nc.sync.dma_start(tg[:], gt_v[:, sl])

                    st = stats[s]
                    jM = 2 * c
                    # lp/l1p first: pred is the first DMA of the chunk, so
                    # ScalarE starts before mask/gt have landed
                    lp = mid.tile([P, CH], bf16, tag="lp", name=f"lp_{s}_{c}")
                    nc.scalar.activation(lp[:], tp[:], Act.Ln)
                    l1p = mid.tile([P, CH], bf16, tag="l1p",
                                   name=f"l1p_{s}_{c}")
                    nc.scalar.activation(l1p[:], tp[:], Act.Ln,
                                         bias=1.0, scale=-1.0)
                    # bf16 "cast" of the 0/1 mask as ln((e-1)*x + 1),
                    # exactly 0->0, 1->1; accum gives the M column for free
                    m16 = mid.tile([P, CH], bf16, tag="m16",
                                   name=f"m16_{s}_{c}")
                    nc.scalar.activation(m16[:], tm[:], Act.Ln,
                                         bias=1.0, scale=float(np.e - 1.0),
                                         accum_out=st[:, jM:jM + 1])
                    gm = mid.tile([P, CH], bf16, tag="gm", name=f"gm_{s}_{c}")
                    nc.vector.tensor_tensor(gm[:], tg[:], tm[:], Alu.mult)
                    nm = mid.tile([P, CH], bf16, tag="nm",
                                  name=f"nm_{s}_{c}")
                    nc.vector.tensor_tensor(nm[:], m16[:], gm[:],
                                            Alu.subtract)
                    t1 = mid.tile([P, CH], bf16, tag="t1", name=f"t1_{s}_{c}")
                    t2 = mid.tile([P, CH], bf16, tag="t2", name=f"t2_{s}_{c}")
                    nc.vector.tensor_tensor(t1[:], gm[:], lp[:], Alu.mult)
                    nc.vector.tensor_tensor(t2[:], nm[:], l1p[:], Alu.mult)

                    # A column: Ln-trick accum pass on ScalarE for sample 0,
                    # free-axis reduce on VectorE for sample 1
                    if s == 0:
                        junk = mid.tile([P, CH], bf16, tag="junk", bufs=1,
                                        name=f"junk_{s}_{c}")
                        nc.scalar.activation(junk[:], gm[:], Act.Ln,
                                             bias=1.0,
                                             scale=float(np.e - 1.0),
                                             accum_out=st[:, jM + 1:jM + 2])
                    else:
                        nc.vector.tensor_reduce(st[:, jM + 1:jM + 2], gm[:],
                                                Ax.X, Alu.add)

                    if bypass_pe:
                        jC = 2 * len(CHUNKS)
                        nc.vector.tensor_reduce(st[:, jC:jC + 1], t1[:],
                                                Ax.X, Alu.add)
                        nc.vector.tensor_reduce(st[:, jC + 1:jC + 2], t2[:],
                                                Ax.X, Alu.add)
                    else:
                        for m in range(CH // MM):
                            nc.tensor.matmul(acc[:], ones[:],
                                             t1[:, bass.ts(m, MM)],
                                             start=step == 0,
                                             stop=step == nsteps - 1)
                            step += 1
                            nc.tensor.matmul(acc[:], ones[:],
                                             t2[:, bass.ts(m, MM)],
                                             start=step == 0,
                                             stop=step == nsteps - 1)
                            step += 1

                # PSUM -> SBUF on ScalarE (it has slack at both points);
                # sample 0's outputs are DMA'd mid-stream, sample 1's at the
                # end -- always from SP after that chunk's input triggers so
                # inputs are never head-of-line blocked
                cd_sb = res.tile([1, MM], f32, tag=f"cd{s}", name=f"cd_sb{s}")
                nc.scalar.copy(cd_sb[:], acc[:])
                if s == 0:
                    nc.sync.dma_start(cd_d[0], cd_sb[:])
                    nc.sync.dma_start(stats0_d[:], stats0[:])
                else:
                    nc.sync.dma_start(cd_d[1], cd_sb[:])
                    nc.sync.dma_start(stats1_d[:], stats1[:])
    nc.compile()
    return nc


def _get_nc():
    if "nc" not in _STATE:
        _STATE["nc"] = _build()
    return _STATE["nc"]


def _host_topk_fallback(p, g, m):
    """Exact per-sample reference semantics in numpy (rare path)."""
    p = p.astype(np.float32)
    positive = g * m
    negative = (1.0 - g) * m
    pos_count = positive.sum(dtype=np.float64)
    neg_count = min(negative.sum(dtype=np.float64), pos_count * NEG_RATIO)
    log_p = np.maximum(np.log(p), -100.0)
    log_1mp = np.maximum(np.log1p(-p), -100.0)
    loss = -(g * log_p + (1.0 - g) * log_1mp)
    pos_loss_sum = (loss * positive).sum(dtype=np.float64)
    neg_loss = (loss * negative).ravel()
    k = int(neg_count)
    if k > 0:
        top = np.partition(neg_loss, len(neg_loss) - k)[len(neg_loss) - k:]
        neg_topk = top.sum(dtype=np.float64)
    else:
        neg_topk = 0.0
    return (pos_loss_sum + neg_topk) / (pos_count + neg_count + EPS)


def _combine(results, p, g, m):
    losses = []
    for c in range(N_CORES):
        cd = results[c]["cd"].astype(np.float64)        # [S, MM]
        for s in range(S):
            st = results[c][f"stats{s}"].astype(np.float64)
            nch = len(CHUNK_PLANS[s])
            M = st[:, 0:2 * nch:2].sum()
            A = st[:, 1:2 * nch:2].sum()
            CD = cd[s].sum()
            if s == S - 1:
                CD += st[:, 2 * nch:2 * nch + 2].sum()
            pos_count = round(A)
            neg_raw = round(M - A)
            if neg_raw <= pos_count * NEG_RATIO:
                # top-k covers every (strictly positive) negative loss
                losses.append((-CD) / (pos_count + neg_raw + EPS))
            else:
                i = c * S + s
                losses.append(_host_topk_fallback(p[i], g[i], m[i]))
    return np.float32(np.mean(losses))


def _in_maps(p, g, m):
    return [
        {"pred": p[c * S:(c + 1) * S],
         "gt": g[c * S:(c + 1) * S],
         "mask": m[c * S:(c + 1) * S]}
        for c in range(N_CORES)
    ]


def kernel(pred, gt, mask):
    from concourse import bass_utils

    p = np.ascontiguousarray(pred[:, 0], dtype=np.float32)   # [N,H,W]
    g = np.ascontiguousarray(gt, dtype=np.float32)
    m = np.ascontiguousarray(mask, dtype=np.float32)

    nc = _get_nc()
    in_maps = _in_maps(p, g, m)
    try:
        res = bass_utils.run_bass_kernel_spmd(nc, in_maps,
                                              core_ids=list(range(N_CORES)))
    except Exception:
        # one retry: transient device wedge from a prior process
        res = bass_utils.run_bass_kernel_spmd(nc, in_maps,
                                              core_ids=list(range(N_CORES)))
    return _combine(res.results, p, g, m)


# revision 13
# speedup vs baseline: 1.1329x; 1.0496x over previous
"""Balanced BCE loss with per-sample dynamic top-k negative mining on 8 TRN2 cores.

Math: for each sample the reference computes
    pos_count = sum(gt*mask), neg_raw = sum((1-gt)*mask)
    neg_count = min(neg_raw, 3*pos_count), k = int(neg_count)
    loss = BCE(pred, gt);  pos_loss = sum(loss*positive)
    neg_topk = sum of k largest loss*negative values
    per_sample = (pos_loss + neg_topk) / (pos_count + neg_count + eps); mean over N.

Every negative position has loss > 0 (p is bounded away from {0,1}), so
whenever neg_raw <= 3*pos_count the top-k sum equals the FULL sum of negative
losses and the per-sample loss reduces to three streaming scalars:
    A = sum(gt*mask)   M = sum(mask)   (B = M - A)
    CD = sum_masked ln(p if gt else 1-p)     (= pos_loss + neg_sum, negated)
If a sample ever violates neg_raw <= 3*pos_count the host recomputes it
exactly (numpy).

Device mapping: data-parallel over N, 2 samples/core.  Each [640,640] sample
is a [128, 3200] view (12800B contiguous per partition), streamed in free-dim
chunks (small first chunk so compute starts early, small last chunk so the
post-DMA dependency tail is short).  Per chunk:
  - ScalarE: lp = Ln(p); l1p = Ln(1-p) (activation scale/bias); m16 = bf16
    "cast" of mask via Ln((e-1)*x+1) (exact 0->0, 1->1, keeps one activation
    table) whose accum_out is the per-chunk M column.
  - VectorE: gm = gt*mask (f32 in, bf16 out); u = m16*l1p (2x bf16) giving
    ln(1-p) on every masked position and 0 elsewhere; then
    copy_predicated(u, gm, lp) overwrites masked positives with ln(p).
    u is now the complete per-element masked log-loss (= C+D contribution)
    in THREE VectorE passes total -- no nm/t1/t2 intermediates.
  - TensorE: two [1,400] PSUM accumulators per sample via a stationary
    ones[128,1] bf16 vector: accCD sums u, accA sums gm (pos_count).
  - The final 400-wide chunk bypasses the PE: VectorE tensor_reduces write
    its A/CD columns directly, keeping the last-chunk chain short.
GpSimd only does the ones[] memset: its software tensor ops contend with
VectorE for SBUF and slow DVE ~4x (measured), so no compute goes there.
Input DMAs are triggered from SP in pred,mask,gt order (ScalarE starts on
pred before gt lands); ALL output DMA triggers are emitted after the last
input trigger so SP's in-order stream never head-of-line blocks input DMAs
behind compute (measured: a mid-stream blocked output trigger stalled the
remaining input stream by ~7us).  Host sums the per-partition/per-chunk
partials in float64; M and A are exact integers (0/1 tensors, f32
accumulators), so B = M - A is exact and the fast/fallback decision is
robust.  Only ln values are rounded to bf16 (~2^-9 relative, averaging out
over ~100k summed elements per sample).
"""

import os
import sys

# defensive: if a previous process left a NeuronCore wedged, ask NRT to
# reset cores at init (read before first jax/NRT touch; harmless otherwise)
os.environ.setdefault("NEURON_RT_RESET_CORES", "1")

if "/opt/trn_rl_repo" not in sys.path:
    sys.path.insert(0, "/opt/trn_rl_repo")

import numpy as np

N, H, W = 16, 640, 640
NEG_RATIO = 3.0
EPS = 1e-8
N_CORES = 8
S = N // N_CORES          # samples per core
P = 128
FREE = H * W // P         # 3200
CHUNK_PLANS = ((800, 1200, 1200), (1600, 1200, 400))
MM = 400                  # matmul sub-chunk (PSUM bank: <=512 f32)
# stats columns: one M column per chunk, plus [A, CD] for the PE-bypass
# final chunk of the last sample
NCOLS = tuple(len(p) + (2 if s == S - 1 else 0)
              for s, p in enumerate(CHUNK_PLANS))

_STATE = {}


def _build():
    import concourse.bass as bass
    import concourse.tile as tile
    from concourse import bacc, mybir

    f32 = mybir.dt.float32
    bf16 = mybir.dt.bfloat16
    Alu = mybir.AluOpType
    Act = mybir.ActivationFunctionType
    Ax = mybir.AxisListType

    nc = bacc.Bacc("TRN2", target_bir_lowering=False, debug=False,
                   num_devices=N_CORES)
    pred_d = nc.dram_tensor("pred", [S, H, W], f32, kind="ExternalInput").ap()
    gt_d = nc.dram_tensor("gt", [S, H, W], f32, kind="ExternalInput").ap()
    mask_d = nc.dram_tensor("mask", [S, H, W], f32, kind="ExternalInput").ap()
    stats0_d = nc.dram_tensor("stats0", [P, NCOLS[0]], f32,
                              kind="ExternalOutput").ap()
    stats1_d = nc.dram_tensor("stats1", [P, NCOLS[1]], f32,
                              kind="ExternalOutput").ap()
    # per sample: [CD partial cols | A partial cols]
    cd_d = nc.dram_tensor("cd", [S, 2 * MM], f32, kind="ExternalOutput").ap()

    with tile.TileContext(nc) as tc:
        with tc.tile_pool(name="cst", bufs=1) as cst, \
             tc.tile_pool(name="inp", bufs=3) as inp, \
             tc.tile_pool(name="mid", bufs=2) as mid, \
             tc.tile_pool(name="res", bufs=1) as res, \
             tc.tile_pool(name="ps", bufs=2, space="PSUM") as psp:
            ones = cst.tile([P, 1], bf16)
            nc.gpsimd.memset(ones[:], 1.0)
            stats0 = res.tile([P, NCOLS[0]], f32)
            stats1 = res.tile([P, NCOLS[1]], f32)
            stats = (stats0, stats1)
            pending_copies = []
            deferred_outs = []

            for s in range(S):
                pred_v = pred_d[s].rearrange("(p a) w -> p (a w)", p=P)
                gt_v = gt_d[s].rearrange("(p a) w -> p (a w)", p=P)
                mask_v = mask_d[s].rearrange("(p a) w -> p (a w)", p=P)
                accCD = psp.tile([1, MM], f32, tag="accCD", name=f"accCD_{s}")
                accA = psp.tile([1, MM], f32, tag="accA", name=f"accA_{s}")
                CHUNKS = CHUNK_PLANS[s]
                pe_chunks = [c for c, ch in enumerate(CHUNKS)
                             if not (s == S - 1 and c == len(CHUNKS) - 1)]
                nsteps = sum(CHUNKS[c] // MM for c in pe_chunks)
                off = 0
                step = 0
                for c, CH in enumerate(CHUNKS):
                    sl = slice(off, off + CH)
                    off += CH
                    bypass_pe = s == S - 1 and c == len(CHUNKS) - 1
                    tp = inp.tile([P, CH], f32, tag="pred",
                                  name=f"tp_{s}_{c}")
                    tm = inp.tile([P, CH], f32, tag="mask",
                                  name=f"tm_{s}_{c}")
                    tg = inp.tile([P, CH], f32, tag="gt", name=f"tg_{s}_{c}")
                    nc.sync.dma_start(tp[:], pred_v[:, sl])
                    nc.sync.dma_start(tm[:], mask_v[:, sl])
                    nc.sync.dma_start(tg[:], gt_v[:, sl])

                    st = stats[s]
                    # lp/l1p first: pred is the first DMA of the chunk, so
                    # ScalarE starts before mask/gt have landed
                    lp = mid.tile([P, CH], bf16, tag="lp", name=f"lp_{s}_{c}")
                    nc.scalar.activation(lp[:], tp[:], Act.Ln)
                    l1p = mid.tile([P, CH], bf16, tag="l1p",
                                   name=f"l1p_{s}_{c}")
                    nc.scalar.activation(l1p[:], tp[:], Act.Ln,
                                         bias=1.0, scale=-1.0)
                    # bf16 "cast" of the 0/1 mask as ln((e-1)*x + 1),
                    # exactly 0->0, 1->1; accum gives the M column for free
                    m16 = mid.tile([P, CH], bf16, tag="m16",
                                   name=f"m16_{s}_{c}")
                    nc.scalar.activation(m16[:], tm[:], Act.Ln,
                                         bias=1.0, scale=float(np.e - 1.0),
                                         accum_out=st[:, c:c + 1])
                    gm = mid.tile([P, CH], bf16, tag="gm", name=f"gm_{s}_{c}")
                    nc.vector.tensor_tensor(gm[:], tg[:], tm[:], Alu.mult)
                    # overwrite l1p in place with lp wherever gt==1: the
                    # predicate is the raw gt tile bitcast to int32
                    # (1.0f == 0x3F800000 != 0), satisfying CopyPredicated's
                    # integer-predicate requirement with zero extra passes
                    nc.vector.copy_predicated(l1p[:],
                                              tg[:].bitcast(mybir.dt.int32),
                                              lp[:])
                    # u = masked chosen-log: ln(p) on masked positives,
                    # ln(1-p) on masked negatives, 0 elsewhere (= t1+t2)
                    u = mid.tile([P, CH], bf16, tag="u", name=f"u_{s}_{c}")
                    nc.vector.tensor_tensor(u[:], m16[:], l1p[:], Alu.mult)

                    if bypass_pe:
                        jA = len(CHUNKS)
                        nc.vector.tensor_reduce(st[:, jA:jA + 1], gm[:],
                                                Ax.X, Alu.add)
                        nc.vector.tensor_reduce(st[:, jA + 1:jA + 2], u[:],
                                                Ax.X, Alu.add)
                    else:
                        for m in range(CH // MM):
                            nc.tensor.matmul(accA[:], ones[:],
                                             gm[:, bass.ts(m, MM)],
                                             start=step == 0,
                                             stop=step == nsteps - 1)
                            nc.tensor.matmul(accCD[:], ones[:],
                                             u[:, bass.ts(m, MM)],
                                             start=step == 0,
                                             stop=step == nsteps - 1)
                            step += 1
                    # emit the previous sample's PSUM->SBUF copies AFTER
                    # this chunk's VectorE work so VectorE never stalls
                    # waiting for the PE to close the accumulators
                    if pending_copies:
                        for dst_sb, acc_ap in pending_copies:
                            nc.vector.tensor_copy(dst_sb, acc_ap)
                        pending_copies = []

                cd_sb = res.tile([1, 2 * MM], f32, tag=f"cd{s}",
                                 name=f"cd_sb{s}")
                pending_copies = [(cd_sb[:, 0:MM], accCD[:]),
                                  (cd_sb[:, MM:2 * MM], accA[:])]
                if s == 0:
                    deferred_outs.append((cd_d[0], cd_sb))
                    deferred_outs.append((stats0_d[:], stats0))
                else:
                    # last sample: flush copies now (no later chunk)
                    for dst_sb, acc_ap in pending_copies:
                        nc.vector.tensor_copy(dst_sb, acc_ap)
                    pending_copies = []
                    deferred_outs.append((cd_d[1], cd_sb))
                    deferred_outs.append((stats1_d[:], stats1))

            for dst, src in deferred_outs:
                nc.sync.dma_start(dst, src[:])
    nc.compile()
    return nc


def _get_nc():
    if "nc" not in _STATE:
        _STATE["nc"] = _build()
    return _STATE["nc"]


def _host_topk_fallback(p, g, m):
    """Exact per-sample reference semantics in numpy (rare path)."""
    p = p.astype(np.float32)
    positive = g * m
    negative = (1.0 - g) * m
    pos_count = positive.sum(dtype=np.float64)
    neg_count = min(negative.sum(dtype=np.float64), pos_count * NEG_RATIO)
    log_p = np.maximum(np.log(p), -100.0)
    log_1mp = np.maximum(np.log1p(-p), -100.0)
    loss = -(g * log_p + (1.0 - g) * log_1mp)
    pos_loss_sum = (loss * positive).sum(dtype=np.float64)
    neg_loss = (loss * negative).ravel()
    k = int(neg_count)
    if k > 0:
        top = np.partition(neg_loss, len(neg_loss) - k)[len(neg_loss) - k:]
        neg_topk = top.sum(dtype=np.float64)
    else:
        neg_topk = 0.0
    return (pos_loss_sum + neg_topk) / (pos_count + neg_count + EPS)


def _combine(results, p, g, m):
    losses = []
    for c in range(N_CORES):
        cd = results[c]["cd"].astype(np.float64)        # [S, 2*MM]
        for s in range(S):
            st = results[c][f"stats{s}"].astype(np.float64)
            nch = len(CHUNK_PLANS[s])
            M = st[:, 0:nch].sum()
            A = cd[s, MM:].sum()
            CD = cd[s, :MM].sum()
            if s == S - 1:
                A += st[:, nch].sum()
                CD += st[:, nch + 1].sum()
            pos_count = round(A)
            neg_raw = round(M - A)
            if neg_raw <= pos_count * NEG_RATIO:
                # top-k covers every (strictly positive) negative loss
                losses.append((-CD) / (pos_count + neg_raw + EPS))
            else:
                i = c * S + s
                losses.append(_host_topk_fallback(p[i], g[i], m[i]))
    return np.float32(np.mean(losses))


def _in_maps(p, g, m):
    return [
        {"pred": p[c * S:(c + 1) * S],
         "gt": g[c * S:(c + 1) * S],
         "mask": m[c * S:(c + 1) * S]}
        for c in range(N_CORES)
    ]


def kernel(pred, gt, mask):
    from concourse import bass_utils

    p = np.ascontiguousarray(pred[:, 0], dtype=np.float32)   # [N,H,W]
    g = np.ascontiguousarray(gt, dtype=np.float32)
    m = np.ascontiguousarray(mask, dtype=np.float32)

    nc = _get_nc()
    in_maps = _in_maps(p, g, m)
    try:
        res = bass_utils.run_bass_kernel_spmd(nc, in_maps,
                                              core_ids=list(range(N_CORES)))
    except Exception:
        # one retry: transient device wedge from a prior process
        res = bass_utils.run_bass_kernel_spmd(nc, in_maps,
                                              core_ids=list(range(N_CORES)))
    return _combine(res.results, p, g, m)
